# revision 15
# baseline (speedup 1.0000x reference)
"""Trainium2 Bass kernel for CombinedLoss (chamfer + density) on 8 NeuronCores.

Problem: B=4, N=M=8192, D=3.
  chamfer = mean_n min_m d2[b,n,m] + mean_m min_n d2[b,n,m],  d2 clamped >= 0
  density = mean |pred_densities|
  total   = chamfer_clipped + 0.1 * density

One-pass strategy (v2): each core computes its half-batch distance matrix
ONCE and extracts BOTH reduction directions from it:
  - 8 cores = 4 batches x 2 halves. Core c handles batch c//2, x-half c%2:
    rows = 4096 pred points (32 tiles x 128), cols = all 8192 target points
    (4 groups x 2048).
  - d2 produced in f32 PSUM by one K=24 matmul per 512-col slice (exact
    bf16-triple split of coords/norms, largest-first accumulation order).
  - ScalarE drains every PSUM group to fp16 SBUF (the only PSUM-port work).
  - VectorE: row-min = 2 pairwise fp16 tensor_tensor mins + 1 fused
    tensor_tensor_reduce (min,min) that emits the per-row min scalar.
  - Column-min partials accumulate in 4 persistent [128,2048] fp16 tiles
    via in-place tensor_tensor mins (VectorE, optionally GpSimd for one
    group). DMA'd out at the end; host finishes the partition-axis min and
    combines the two half cores, so the y->x chamfer direction needs no
    second matmul pass: drained PSUM volume is halved vs the 2-pass design.
  - Host: clamp mins at 0, means in f64, assemble the three scalars.
"""

import os
from contextlib import ExitStack

import ml_dtypes
import numpy as np

import concourse.tile as tile
from concourse import bacc, mybir
from concourse.bass_utils import run_bass_kernel_spmd

B, N, M, D = 4, 8192, 8192, 3
R = N // 2          # rows per core
NT = R // 128       # 32 row tiles
NG = M // 2048      # 4 column groups
K = 24              # contraction rows of the distance matmul

BF16 = ml_dtypes.bfloat16
FP16INF = 60000.0   # > max d2 (~50), < fp16 max

# One-pass modes. "pp" = ping-pong colmin accumulators (distinct tt output
# each tile keeps the DVE in 2x fp16 mode; in-place accumulation drops to 1x).
# "f" suffix: fold-tree rowmin tail instead of a full-width 1x tensor_reduce.
# Legacy: "opn" (in-place colmin, plain tail), "op" (TTR tail; crashes HW).
MODE = os.environ.get("CHAMFER_MODE", "ppf")


def _split3(a_f64):
    """Split values into 3 bf16 parts summing (near-)exactly to the input."""
    p0 = a_f64.astype(BF16)
    r1 = a_f64 - p0.astype(np.float64)
    p1 = r1.astype(BF16)
    r2 = r1 - p1.astype(np.float64)
    p2 = r2.astype(BF16)
    return p0, p1, p2


def _build_operands(rows_pts, cols_pts):
    """Stationary [K, R] and moving [K, ncols] bf16 matrices so that
    (stat.T @ mov)[i, j] = ||rows_pts[i] - cols_pts[j]||^2 in f32-grade accuracy.
    """
    a = rows_pts.astype(np.float64)
    b = cols_pts.astype(np.float64)
    a2 = (a * a).sum(-1)
    b2 = (b * b).sum(-1)
    ah, am, al = _split3(a.T)      # each [3, R]
    bh, bm, bl = _split3(b.T)      # each [3, ncols]
    a2h, a2m, a2l = _split3(a2)    # [R]
    b2h, b2m, b2l = _split3(b2)    # [ncols]

    nr, ncols = a.shape[0], b.shape[0]
    S = np.zeros((K, nr), BF16)
    Mv = np.zeros((K, ncols), BF16)
    ones_r = np.ones((nr,), BF16)
    ones_c = np.ones((ncols,), BF16)

    def neg2(t):
        return (-2.0 * t.astype(np.float32)).astype(BF16)  # exact for bf16 input

    # rows ordered largest magnitude first for benign psum accumulation order
    S[0], Mv[0] = a2h, ones_c
    S[1], Mv[1] = ones_r, b2h
    S[2:5], Mv[2:5] = neg2(ah), bh          # hh
    S[5], Mv[5] = a2m, ones_c
    S[6], Mv[6] = ones_r, b2m
    S[7:10], Mv[7:10] = neg2(ah), bm        # hm
    S[10:13], Mv[10:13] = neg2(am), bh      # mh
    S[13], Mv[13] = a2l, ones_c
    S[14], Mv[14] = ones_r, b2l
    S[15:18], Mv[15:18] = neg2(ah), bl      # hl
    S[18:21], Mv[18:21] = neg2(al), bh      # lh
    S[21:24], Mv[21:24] = neg2(am), bm      # mm
    return S, Mv


def _emit_onepass(ctx, tc, pools, stat_ap, mov_ap, row_ap, col_ap, mode):
    nc = tc.nc
    big, psum, small = pools
    f32 = mybir.dt.float32
    bf16 = mybir.dt.bfloat16
    fp16 = mybir.dt.float16
    MIN = mybir.AluOpType.min
    X = mybir.AxisListType.X

    mov_sb = big.tile([K, M], bf16, tag="mov")
    stat_sb = big.tile([K, R], bf16, tag="stat")
    for c in range(4):
        nc.sync.dma_start(
            mov_sb[:, c * (M // 4) : (c + 1) * (M // 4)],
            mov_ap[:, c * (M // 4) : (c + 1) * (M // 4)],
        )
    for c in range(2):
        nc.sync.dma_start(
            stat_sb[:, c * (R // 2) : (c + 1) * (R // 2)],
            stat_ap[:, c * (R // 2) : (c + 1) * (R // 2)],
        )
    rowred = big.tile([128, NT], f32, tag="rowred")
    pingpong = mode.startswith("pp")
    wide = "w" in mode
    nrun = 2 if pingpong else 1
    ngrp = NG // 2 if wide else NG  # run/cp tile count per set
    gw = 4096 if wide else 2048     # run/cp tile width
    runs = [
        [
            big.tile([128, gw], fp16, name=f"run{g}_{p}", tag=f"run{g}_{p}")
            for p in range(nrun)
        ]
        for g in range(ngrp)
    ]
    if not pingpong:
        for g in range(ngrp):
            nc.vector.memset(runs[g][0][:], FP16INF)

    def fill(ps, t, g):
        for s in range(4):
            nc.tensor.matmul(
                ps[:, 512 * s : 512 * (s + 1)],
                lhsT=stat_sb[:, 128 * t : 128 * (t + 1)],
                rhs=mov_sb[:, 2048 * g + 512 * s : 2048 * g + 512 * (s + 1)],
                start=True,
                stop=True,
            )

    for t in range(NT):
        if wide:
            cpw = []
            for gg in range(2):
                cp = small.tile([128, 4096], fp16, tag=f"cpw{gg}", bufs=2)
                for k in range(2):
                    ps = psum.tile([128, 2048], f32, tag="ps")
                    fill(ps, t, 2 * gg + k)
                    nc.scalar.copy(cp[:, 2048 * k : 2048 * (k + 1)], ps[:])
                cpw.append(cp)
            cps = [cpw[0][:, 0:2048], cpw[0][:, 2048:4096],
                   cpw[1][:, 0:2048], cpw[1][:, 2048:4096]]
        else:
            cpw = None
            cps = []
            for g in range(NG):
                ps = psum.tile([128, 2048], f32, tag="ps")
                fill(ps, t, g)
                cp = small.tile([128, 2048], fp16, tag=f"cp{g}", bufs=3)
                nc.scalar.copy(cp[:], ps[:])
                cps.append(cp[:])

        def colmin(g):
            src = cpw[g][:] if wide else cps[g]
            nxt = runs[g][t % nrun]
            if pingpong and t == 0:
                nc.vector.tensor_copy(nxt[:], src)
            else:
                prv = runs[g][(t - 1) % nrun]
                nc.vector.tensor_tensor(nxt[:], prv[:], src, op=MIN)

        # DVE op order interleaved with ScalarE copy arrival: colmin(g) only
        # needs cp_g, the tree ops need pairs.
        colmin(0)
        h01 = small.tile([128, 2048], fp16, tag="h01", bufs=1)
        nc.vector.tensor_tensor(h01[:], cps[0], cps[1], op=MIN)
        if not wide:
            colmin(1)
            colmin(2)
        h23 = small.tile([128, 2048], fp16, tag="h23", bufs=1)
        nc.vector.tensor_tensor(h23[:], cps[2], cps[3], op=MIN)
        colmin(3 if not wide else 1)
        h = small.tile([128, 2048], fp16, tag="h", bufs=1)
        nc.vector.tensor_tensor(h[:], h01[:], h23[:], op=MIN)
        if "f" in mode:
            # fold 2048 -> 128 with 2x-rate tts, then one short 1x reduce
            w = 1024
            src = h
            while w >= 128:
                q = small.tile([128, w], fp16, tag=f"q{w}", bufs=1)
                nc.vector.tensor_tensor(q[:], src[:, 0:w], src[:, w : 2 * w], op=MIN)
                src = q
                w //= 2
            nc.vector.tensor_reduce(rowred[:, t : t + 1], src[:], axis=X, op=MIN)
        else:
            nc.vector.tensor_reduce(rowred[:, t : t + 1], h[:], axis=X, op=MIN)

    nc.sync.dma_start(row_ap[:], rowred[:])
    ngw = NG // ngrp
    for g in range(ngrp):
        nc.sync.dma_start(
            col_ap[:, g * ngw : (g + 1) * ngw, :], runs[g][(NT - 1) % nrun][:]
        )


def _build_program(rep: int = 1, mode: str | None = None):
    mode = MODE if mode is None else mode
    nc = bacc.Bacc("TRN2", target_bir_lowering=False, debug=False, num_devices=8)
    bf16 = mybir.dt.bfloat16
    f32 = mybir.dt.float32
    fp16 = mybir.dt.float16
    statA = nc.dram_tensor("statA", [K, R], bf16, kind="ExternalInput").ap()
    movA = nc.dram_tensor("movA", [K, M], bf16, kind="ExternalInput").ap()
    rowA = nc.dram_tensor("rowA", [128, NT], f32, kind="ExternalOutput").ap()
    colA = nc.dram_tensor("colA", [128, NG, 2048], fp16, kind="ExternalOutput").ap()

    with tile.TileContext(nc) as tc:
        with ExitStack() as ctx:
            big = ctx.enter_context(tc.tile_pool(name="big", bufs=1))
            psum = ctx.enter_context(tc.tile_pool(name="psum", bufs=2, space="PSUM"))
            small = ctx.enter_context(tc.tile_pool(name="small", bufs=2))
            pools = (big, psum, small)

            def body(_i=None):
                _emit_onepass(ctx, tc, pools, statA, movA, rowA, colA, mode)

            if rep == 1:
                body()
            else:
                with tc.For_i(0, rep, 1) as i:
                    body(i)
    nc.compile()
    return nc


_NC_CACHE = None


def _get_program():
    global _NC_CACHE
    if _NC_CACHE is None:
        _NC_CACHE = _build_program()
    return _NC_CACHE


def _decode_rowmin(arr):
    # arr [128, NT] with value for local row t*128+p at [p, t]
    return arr.T.reshape(R)


def _make_in_maps(pred_points, target_points):
    in_maps = []
    for c in range(8):
        b, h = divmod(c, 2)
        x_half = pred_points[b, h * R : (h + 1) * R]
        SA, MA = _build_operands(x_half, target_points[b])
        in_maps.append({"statA": SA, "movA": MA})
    return in_maps


def kernel(pred_points, target_points, pred_densities):
    pred_points = np.asarray(pred_points, np.float32)
    target_points = np.asarray(target_points, np.float32)
    pred_densities = np.asarray(pred_densities, np.float32)

    nc = _get_program()
    in_maps = _make_in_maps(pred_points, target_points)
    res = run_bass_kernel_spmd(nc, in_maps, core_ids=list(range(8)))

    mins_x = np.empty((B, N), np.float64)
    mins_y = np.full((B, M), np.inf)
    for c in range(8):
        b, h = divmod(c, 2)
        mins_x[b, h * R : (h + 1) * R] = _decode_rowmin(res.results[c]["rowA"])
        # colA: [128, NG, 2048] fp16; col j = g*2048 + cc; min over partitions
        colpart = (
            res.results[c]["colA"].astype(np.float32).min(axis=0).reshape(M)
        )
        mins_y[b] = np.minimum(mins_y[b], colpart)

    cham_x = np.maximum(mins_x, 0.0).mean()
    cham_y = np.maximum(mins_y, 0.0).mean()
    chamfer = np.clip(cham_x + cham_y, 0.0, 1.0e6)
    density = np.abs(pred_densities.astype(np.float64)).mean()
    total = 1.0 * chamfer + 0.1 * density
    return (
        np.float32(total),
        np.float32(chamfer),
        np.float32(density),
    )


# revision 19
# speedup vs baseline: 3.5619x; 3.5619x over previous
"""Trainium2 Bass kernel for CombinedLoss (chamfer + density) on 8 NeuronCores.

Problem: B=4, N=M=8192, D=3.
  chamfer = mean_n min_m d2[b,n,m] + mean_m min_n d2[b,n,m],  d2 clamped >= 0
  density = mean |pred_densities|
  total   = chamfer_clipped + 0.1 * density

One-pass strategy (v2): each core computes its half-batch distance matrix
ONCE and extracts BOTH reduction directions from it:
  - 8 cores = 4 batches x 2 halves. Core c handles batch c//2, x-half c%2:
    rows = 4096 pred points (32 tiles x 128), cols = all 8192 target points
    (4 groups x 2048).
  - d2 produced in f32 PSUM by one K=24 matmul per 512-col slice (exact
    bf16-triple split of coords/norms, largest-first accumulation order).
  - ScalarE drains every PSUM group to fp16 SBUF (the only PSUM-port work).
  - VectorE: row-min = 2 pairwise fp16 tensor_tensor mins + 1 fused
    tensor_tensor_reduce (min,min) that emits the per-row min scalar.
  - Column-min partials accumulate in 4 persistent [128,2048] fp16 tiles
    via in-place tensor_tensor mins (VectorE, optionally GpSimd for one
    group). DMA'd out at the end; host finishes the partition-axis min and
    combines the two half cores, so the y->x chamfer direction needs no
    second matmul pass: drained PSUM volume is halved vs the 2-pass design.
  - Host: clamp mins at 0, means in f64, assemble the three scalars.
"""

import os
from contextlib import ExitStack

import ml_dtypes
import numpy as np

import concourse.tile as tile
from concourse import bacc, mybir
from concourse.bass_utils import run_bass_kernel_spmd

B, N, M, D = 4, 8192, 8192, 3
R = N // 2          # rows per core
NT = R // 128       # 32 row tiles
NG = M // 2048      # 4 column groups
K = 24              # contraction rows of the distance matmul

BF16 = ml_dtypes.bfloat16
FP16INF = 60000.0   # > max d2 (~50), < fp16 max

# One-pass modes. "pp" = ping-pong colmin accumulators (distinct tt output
# each tile keeps the DVE in 2x fp16 mode; in-place accumulation drops to 1x).
# "f" suffix: fold-tree rowmin tail instead of a full-width 1x tensor_reduce.
# Legacy: "opn" (in-place colmin, plain tail), "op" (TTR tail; crashes HW).
MODE = os.environ.get("CHAMFER_MODE", "ppf")


def _split3(a_f64):
    """Split values into 3 bf16 parts summing (near-)exactly to the input."""
    p0 = a_f64.astype(BF16)
    r1 = a_f64 - p0.astype(np.float64)
    p1 = r1.astype(BF16)
    r2 = r1 - p1.astype(np.float64)
    p2 = r2.astype(BF16)
    return p0, p1, p2


def _build_operands(rows_pts, cols_pts):
    """Stationary [K, R] and moving [K, ncols] bf16 matrices so that
    (stat.T @ mov)[i, j] = ||rows_pts[i] - cols_pts[j]||^2 in f32-grade accuracy.
    """
    a = rows_pts.astype(np.float64)
    b = cols_pts.astype(np.float64)
    a2 = (a * a).sum(-1)
    b2 = (b * b).sum(-1)
    ah, am, al = _split3(a.T)      # each [3, R]
    bh, bm, bl = _split3(b.T)      # each [3, ncols]
    a2h, a2m, a2l = _split3(a2)    # [R]
    b2h, b2m, b2l = _split3(b2)    # [ncols]

    nr, ncols = a.shape[0], b.shape[0]
    S = np.zeros((K, nr), BF16)
    Mv = np.zeros((K, ncols), BF16)
    ones_r = np.ones((nr,), BF16)
    ones_c = np.ones((ncols,), BF16)

    def neg2(t):
        return (-2.0 * t.astype(np.float32)).astype(BF16)  # exact for bf16 input

    # rows ordered largest magnitude first for benign psum accumulation order
    S[0], Mv[0] = a2h, ones_c
    S[1], Mv[1] = ones_r, b2h
    S[2:5], Mv[2:5] = neg2(ah), bh          # hh
    S[5], Mv[5] = a2m, ones_c
    S[6], Mv[6] = ones_r, b2m
    S[7:10], Mv[7:10] = neg2(ah), bm        # hm
    S[10:13], Mv[10:13] = neg2(am), bh      # mh
    S[13], Mv[13] = a2l, ones_c
    S[14], Mv[14] = ones_r, b2l
    S[15:18], Mv[15:18] = neg2(ah), bl      # hl
    S[18:21], Mv[18:21] = neg2(al), bh      # lh
    S[21:24], Mv[21:24] = neg2(am), bm      # mm
    return S, Mv


def _emit_onepass(ctx, tc, pools, stat_ap, mov_ap, row_ap, col_ap, mode):
    nc = tc.nc
    big, psum, small = pools
    f32 = mybir.dt.float32
    bf16 = mybir.dt.bfloat16
    fp16 = mybir.dt.float16
    MIN = mybir.AluOpType.min
    X = mybir.AxisListType.X

    mov_sb = big.tile([K, M], bf16, tag="mov")
    stat_sb = big.tile([K, R], bf16, tag="stat")
    for c in range(4):
        nc.sync.dma_start(
            mov_sb[:, c * (M // 4) : (c + 1) * (M // 4)],
            mov_ap[:, c * (M // 4) : (c + 1) * (M // 4)],
        )
    for c in range(2):
        nc.sync.dma_start(
            stat_sb[:, c * (R // 2) : (c + 1) * (R // 2)],
            stat_ap[:, c * (R // 2) : (c + 1) * (R // 2)],
        )
    rowred = big.tile([128, NT], f32, tag="rowred")
    pingpong = mode.startswith("pp")
    wide = "w" in mode
    nrun = 2 if pingpong else 1
    ngrp = NG // 2 if wide else NG  # run/cp tile count per set
    gw = 4096 if wide else 2048     # run/cp tile width
    runs = [
        [
            big.tile([128, gw], fp16, name=f"run{g}_{p}", tag=f"run{g}_{p}")
            for p in range(nrun)
        ]
        for g in range(ngrp)
    ]
    for g in range(ngrp):
        nc.vector.memset(runs[g][(NT - 1) % nrun][:], FP16INF)

    def fill(ps, t, g):
        for s in range(4):
            nc.tensor.matmul(
                ps[:, 512 * s : 512 * (s + 1)],
                lhsT=stat_sb[:, 128 * t : 128 * (t + 1)],
                rhs=mov_sb[:, 2048 * g + 512 * s : 2048 * g + 512 * (s + 1)],
                start=True,
                stop=True,
            )

    for t in range(NT):
        if wide:
            cpw = []
            for gg in range(2):
                cp = small.tile([128, 4096], fp16, tag=f"cpw{gg}", bufs=2)
                for k in range(2):
                    ps = psum.tile([128, 2048], f32, tag="ps")
                    fill(ps, t, 2 * gg + k)
                    nc.scalar.copy(cp[:, 2048 * k : 2048 * (k + 1)], ps[:])
                cpw.append(cp)
            cps = [cpw[0][:, 0:2048], cpw[0][:, 2048:4096],
                   cpw[1][:, 0:2048], cpw[1][:, 2048:4096]]
        else:
            cpw = None
            cps = []
            for g in range(NG):
                ps = psum.tile([128, 2048], f32, tag="ps")
                fill(ps, t, g)
                cp = small.tile([128, 2048], fp16, tag=f"cp{g}", bufs=2)
                nc.scalar.copy(cp[:], ps[:])
                cps.append(cp[:])

        def colmin(g):
            src = cpw[g][:] if wide else cps[g]
            prv = runs[g][(t - 1) % nrun]
            nxt = runs[g][t % nrun]
            nc.vector.tensor_tensor(nxt[:], prv[:], src, op=MIN)

        # DVE op order interleaved with ScalarE copy arrival: colmin(g) only
        # needs cp_g, the tree ops need pairs.
        h01 = small.tile([128, 2048], fp16, tag="h01")
        nc.vector.tensor_tensor(h01[:], cps[0], cps[1], op=MIN)
        h23 = small.tile([128, 2048], fp16, tag="h23")
        nc.vector.tensor_tensor(h23[:], cps[2], cps[3], op=MIN)
        h = small.tile([128, 2048], fp16, tag="h")
        nc.vector.tensor_tensor(h[:], h01[:], h23[:], op=MIN)
        if "f" in mode:
            # fold 2048 -> 128 with 2x-rate tts, then one short 1x reduce
            w = 1024
            src = h
            while w >= 128:
                q = small.tile([128, w], fp16, tag=f"q{w}")
                nc.vector.tensor_tensor(q[:], src[:, 0:w], src[:, w : 2 * w], op=MIN)
                src = q
                w //= 2
            nc.vector.tensor_reduce(rowred[:, t : t + 1], src[:], axis=X, op=MIN)
        else:
            nc.vector.tensor_reduce(rowred[:, t : t + 1], h[:], axis=X, op=MIN)
        for g in range(ngrp):
            colmin(g)

    nc.sync.dma_start(row_ap[:], rowred[:])
    ngw = NG // ngrp
    for g in range(ngrp):
        nc.sync.dma_start(
            col_ap[:, g * ngw : (g + 1) * ngw, :], runs[g][(NT - 1) % nrun][:]
        )


def _build_program(rep: int = 1, mode: str | None = None):
    mode = MODE if mode is None else mode
    nc = bacc.Bacc("TRN2", target_bir_lowering=False, debug=False, num_devices=8)
    bf16 = mybir.dt.bfloat16
    f32 = mybir.dt.float32
    fp16 = mybir.dt.float16
    statA = nc.dram_tensor("statA", [K, R], bf16, kind="ExternalInput").ap()
    movA = nc.dram_tensor("movA", [K, M], bf16, kind="ExternalInput").ap()
    rowA = nc.dram_tensor("rowA", [128, NT], f32, kind="ExternalOutput").ap()
    colA = nc.dram_tensor("colA", [128, NG, 2048], fp16, kind="ExternalOutput").ap()

    with tile.TileContext(nc) as tc:
        with ExitStack() as ctx:
            big = ctx.enter_context(tc.tile_pool(name="big", bufs=1))
            psum = ctx.enter_context(tc.tile_pool(name="psum", bufs=2, space="PSUM"))
            small = ctx.enter_context(tc.tile_pool(name="small", bufs=2))
            pools = (big, psum, small)

            def body(_i=None):
                _emit_onepass(ctx, tc, pools, statA, movA, rowA, colA, mode)

            if rep == 1:
                body()
            else:
                with tc.For_i(0, rep, 1) as i:
                    body(i)
    nc.compile()
    return nc


_NC_CACHE = None


def _get_program():
    global _NC_CACHE
    if _NC_CACHE is None:
        _NC_CACHE = _build_program()
    return _NC_CACHE


def _decode_rowmin(arr):
    # arr [128, NT] with value for local row t*128+p at [p, t]
    return arr.T.reshape(R)


def _make_in_maps(pred_points, target_points):
    in_maps = []
    for c in range(8):
        b, h = divmod(c, 2)
        x_half = pred_points[b, h * R : (h + 1) * R]
        SA, MA = _build_operands(x_half, target_points[b])
        in_maps.append({"statA": SA, "movA": MA})
    return in_maps


def kernel(pred_points, target_points, pred_densities):
    pred_points = np.asarray(pred_points, np.float32)
    target_points = np.asarray(target_points, np.float32)
    pred_densities = np.asarray(pred_densities, np.float32)

    nc = _get_program()
    in_maps = _make_in_maps(pred_points, target_points)
    res = run_bass_kernel_spmd(nc, in_maps, core_ids=list(range(8)))

    mins_x = np.empty((B, N), np.float64)
    mins_y = np.full((B, M), np.inf)
    for c in range(8):
        b, h = divmod(c, 2)
        mins_x[b, h * R : (h + 1) * R] = _decode_rowmin(res.results[c]["rowA"])
        # colA: [128, NG, 2048] fp16; col j = g*2048 + cc; min over partitions
        colpart = (
            res.results[c]["colA"].astype(np.float32).min(axis=0).reshape(M)
        )
        mins_y[b] = np.minimum(mins_y[b], colpart)

    cham_x = np.maximum(mins_x, 0.0).mean()
    cham_y = np.maximum(mins_y, 0.0).mean()
    chamfer = np.clip(cham_x + cham_y, 0.0, 1.0e6)
    density = np.abs(pred_densities.astype(np.float64)).mean()
    total = 1.0 * chamfer + 0.1 * density
    return (
        np.float32(total),
        np.float32(chamfer),
        np.float32(density),
    )
